# revision 5
# baseline (speedup 1.0000x reference)
"""MoE top-2 routing kernel for Trainium2, 8 NeuronCores, batch-sharded.

Math (per token): logits = x@gate_W + gate_b; top-2 + softmax -> comb[B,E];
h = relu(x@W1[e]+b1[e]); y = h@W2[e]+b2[e]; out = sum_e comb[:,e]*y_e.

v4 changes vs baseline:
 - x is uploaded twice, as fp32 (exact gating; top-2 selection is
   order-sensitive so the gating path must stay bit-identical to the
   baseline's fp32 matmuls) and as f32r for the expert MLP -- this removes
   the F32->F32R DVE conversion passes at the cost of extra DMA, which has
   ample headroom
 - software pipelining: gating/top-2 for chunk c+1 runs while the expert
   MLP for chunk c occupies the PE, so the PE never stalls on the DVE
   top-2 chain
 - comb transpose uses a bf16 identity (1 cycle/row instead of 4)
 - the hidden layout is expert-interleaved (h-row = h*16 + e) so one
   [128, EH-slice] of the broadcast selector serves every h-tile: a single
   SEL matmul per chunk broadcasts combT to all 128 h-rows
 - relu and the final output copy ride the otherwise-idle ACT engine
"""

import sys
import numpy as np

for _p in ("/opt/trn_rl_repo", "/root/.axon_site/_ro/trn_rl_repo"):
    if _p not in sys.path:
        sys.path.append(_p)

import concourse.bass as bass
import concourse.tile as tile
from concourse import bacc, mybir
from concourse.bass_utils import run_bass_kernel_spmd

F32 = mybir.dt.float32
F32R = mybir.dt.float32r
BF16 = mybir.dt.bfloat16
F16 = mybir.dt.float16
ALU = mybir.AluOpType
ACTF = mybir.ActivationFunctionType

NCORES = 8
B, D, E, H, O = 65536, 784, 16, 64, 10
BL = B // NCORES            # 8192 tokens per core
DP = D + 1                  # 785: ones row appended for bias
EH = E * H                  # 1024
CH = 512                    # tokens per chunk
NCHUNK = BL // CH           # 16
# contraction chunks over DP: six of 128 plus one of 17
KCH = [(i * 128, 128) for i in range(6)] + [(768, DP - 768)]
NK = len(KCH)
NH = EH // 128              # 8 h-col chunks of 128

_CACHED = {}


def _build_program(loop_reps=1):
    nc = bacc.Bacc("TRN2", target_bir_lowering=False, debug=False,
                   num_devices=NCORES)
    xAh_d = nc.dram_tensor("xAh", [NCHUNK, 128, 6 * CH], F16, kind="ExternalInput").ap()
    xBh_d = nc.dram_tensor("xBh", [NCHUNK, DP - 768, CH], F16, kind="ExternalInput").ap()
    xAl_d = nc.dram_tensor("xAl", [NCHUNK, 128, 6 * CH], F16, kind="ExternalInput").ap()
    xBl_d = nc.dram_tensor("xBl", [NCHUNK, DP - 768, CH], F16, kind="ExternalInput").ap()
    Wgh_d = nc.dram_tensor("Wgh", [DP, E], F16, kind="ExternalInput").ap()
    Wgl_d = nc.dram_tensor("Wgl", [DP, E], F16, kind="ExternalInput").ap()
    W1_d = nc.dram_tensor("W1a", [DP, EH], F16, kind="ExternalInput").ap()
    W2_d = nc.dram_tensor("W2a", [EH + E, O], F32R, kind="ExternalInput").ap()
    SEL_d = nc.dram_tensor("SEL", [E, EH], F32R, kind="ExternalInput").ap()
    I16_d = nc.dram_tensor("I16", [E, E], F32, kind="ExternalInput").ap()
    I128_d = nc.dram_tensor("I128", [128, 128], BF16, kind="ExternalInput").ap()
    out_d = nc.dram_tensor("out", [O, BL], F32, kind="ExternalOutput").ap()

    with tile.TileContext(nc) as tc:
        import contextlib
        with contextlib.ExitStack() as ctx:
            wp = ctx.enter_context(tc.tile_pool(name="weights", bufs=1))
            xp = ctx.enter_context(tc.tile_pool(name="xtiles", bufs=2))
            xq = ctx.enter_context(tc.tile_pool(name="xgate", bufs=2))
            sp = ctx.enter_context(tc.tile_pool(name="work", bufs=2))
            ps_a = ctx.enter_context(tc.tile_pool(name="ps_a", bufs=2, space="PSUM"))
            ps_b = ctx.enter_context(tc.tile_pool(name="ps_b", bufs=1, space="PSUM"))
            ps_c = ctx.enter_context(tc.tile_pool(name="ps_c", bufs=1, space="PSUM"))
            ps_e = ctx.enter_context(tc.tile_pool(name="ps_e", bufs=1, space="PSUM"))
            ps_g = ctx.enter_context(tc.tile_pool(name="ps_g", bufs=1, space="PSUM"))
            ps_h = ctx.enter_context(tc.tile_pool(name="ps_h", bufs=2, space="PSUM"))

            # ---- load weights/constants once ----
            Wgh_t, Wgl_t, W1_t = [], [], []
            for k, (s, sz) in enumerate(KCH):
                wgh = wp.tile([sz, E], F16, tag=f"wgh{k}")
                nc.sync.dma_start(wgh[:], Wgh_d[s:s + sz, :])
                Wgh_t.append(wgh)
                wgl = wp.tile([sz, E], F16, tag=f"wgl{k}")
                nc.sync.dma_start(wgl[:], Wgl_d[s:s + sz, :])
                Wgl_t.append(wgl)
                w1 = wp.tile([sz, EH], F16, tag=f"w1{k}")
                nc.sync.dma_start(w1[:], W1_d[s:s + sz, :])
                W1_t.append(w1)
            W2_t = []
            for n in range(NH):
                w2 = wp.tile([128, O], F32R, tag=f"w2{n}")
                nc.sync.dma_start(w2[:], W2_d[n * 128:(n + 1) * 128, :])
                W2_t.append(w2)
            W2b = wp.tile([E, O], F32R, tag="w2b")
            nc.sync.dma_start(W2b[:], W2_d[EH:EH + E, :])
            SEL_t = wp.tile([E, EH], F32R, tag="sel")
            nc.sync.dma_start(SEL_t[:], SEL_d[:])
            I16_t = wp.tile([E, E], F32, tag="i16")
            nc.sync.dma_start(I16_t[:], I16_d[:])
            I128_t = wp.tile([128, 128], BF16, tag="i128")
            nc.sync.dma_start(I128_t[:], I128_d[:])

            def front(c):
                """Load x, exact-fp32 gating, top-2 chain -> comb (bf16)."""
                tAh = xq.tile([128, 6 * CH], F16, tag="tAh")
                nc.sync.dma_start(tAh[:], xAh_d[c])
                tBh = xq.tile([DP - 768, CH], F16, tag="tBh")
                nc.sync.dma_start(tBh[:], xBh_d[c])
                tAl = xp.tile([128, 6 * CH], F16, tag="tAl")
                nc.sync.dma_start(tAl[:], xAl_d[c])
                tBl = xp.tile([DP - 768, CH], F16, tag="tBl")
                nc.sync.dma_start(tBl[:], xBl_d[c])
                xt = [tAh[:, k * CH:(k + 1) * CH] for k in range(6)] + [tBh[:]]
                xl = [tAl[:, k * CH:(k + 1) * CH] for k in range(6)] + [tBl[:]]
                xtr = xt

                # gating: logits = x@Wg to ~2^-22 via fp16 hi/lo 3-term split.
                # hi*hi accumulates in pg; hi*lo and lo*hi (both pre-scaled by
                # 2^11 host-side to dodge fp16 subnormal flush) accumulate in
                # pglo, folded back with one fused multiply-add.
                pg = ps_a.tile([E, CH], F32, tag="pa")
                for k in range(NK):
                    nc.tensor.matmul(pg[:], Wgh_t[k][:], xt[k],
                                     start=(k == 0), stop=(k == NK - 1))
                pglo = ps_g.tile([E, CH], F32, tag="pglo")
                for k in range(NK):
                    nc.tensor.matmul(pglo[:], Wgl_t[k][:], xt[k],
                                     start=(k == 0), stop=False)
                for k in range(NK):
                    nc.tensor.matmul(pglo[:], Wgh_t[k][:], xl[k],
                                     start=False, stop=(k == NK - 1))
                pgls = sp.tile([E, CH], F32, tag="pgls")
                nc.vector.tensor_copy(pgls[:], pglo[:])
                lgT = sp.tile([E, CH], F32, tag="lgT")
                nc.vector.scalar_tensor_tensor(lgT[:], pgls[:], 2.0 ** -11,
                                               pg[:], op0=ALU.mult, op1=ALU.add)
                # transpose to [128, 4*16] via matmul with I16
                pl = ps_b.tile([128, 4 * E], F32, tag="pb")
                for j in range(4):
                    nc.tensor.matmul(pl[:, j * E:(j + 1) * E],
                                     lgT[:, j * 128:(j + 1) * 128],
                                     I16_t[:], start=True, stop=True)
                lg = sp.tile([128, 4 * E], F32, tag="lg")
                nc.vector.tensor_copy(lg[:], pl[:])

                # top-2 + softmax weights -> comb [128, 4, 16]
                lg3 = lg[:].rearrange("p (a e) -> p a e", e=E)
                m1 = sp.tile([128, 4], F32, tag="m1")
                nc.vector.tensor_reduce(m1[:], lg3, axis=mybir.AxisListType.X,
                                        op=ALU.max)
                m1b = m1[:].broadcast_to([128, 4, E])
                ind1 = sp.tile([128, 4 * E], F32, tag="ind1")
                ind1_3 = ind1[:].rearrange("p (a e) -> p a e", e=E)
                nc.vector.tensor_tensor(ind1_3, lg3, m1b, op=ALU.is_equal)
                msk = sp.tile([128, 4 * E], F32, tag="msk")
                msk3 = msk[:].rearrange("p (a e) -> p a e", e=E)
                nc.vector.scalar_tensor_tensor(msk3, ind1_3, -1e30, lg3,
                                               op0=ALU.mult, op1=ALU.add)
                m2 = sp.tile([128, 4], F32, tag="m2")
                nc.vector.tensor_reduce(m2[:], msk3, axis=mybir.AxisListType.X,
                                        op=ALU.max)
                m2b = m2[:].broadcast_to([128, 4, E])
                ind2 = sp.tile([128, 4 * E], F32, tag="ind2")
                ind2_3 = ind2[:].rearrange("p (a e) -> p a e", e=E)
                nc.vector.tensor_tensor(ind2_3, msk3, m2b, op=ALU.is_equal)
                dd = sp.tile([128, 4], F32, tag="dd")
                nc.vector.tensor_tensor(dd[:], m2[:], m1[:], op=ALU.subtract)
                w2s = sp.tile([128, 4], F32, tag="w2s")
                nc.scalar.activation(w2s[:], dd[:], ACTF.Sigmoid)
                w1s = sp.tile([128, 4], F32, tag="w1s")
                nc.vector.tensor_scalar(w1s[:], w2s[:], -1.0, 1.0,
                                        op0=ALU.mult, op1=ALU.add)
                w1b = w1s[:].broadcast_to([128, 4, E])
                w2b_ = w2s[:].broadcast_to([128, 4, E])
                comb = sp.tile([128, 4 * E], BF16, tag="comb")
                comb3 = comb[:].rearrange("p (a e) -> p a e", e=E)
                nc.vector.tensor_tensor(comb3, ind1_3, w1b, op=ALU.mult)
                c2 = sp.tile([128, 4 * E], BF16, tag="c2")
                c2_3 = c2[:].rearrange("p (a e) -> p a e", e=E)
                nc.vector.tensor_tensor(c2_3, ind2_3, w2b_, op=ALU.mult)
                nc.vector.tensor_tensor(comb[:], comb[:], c2[:], op=ALU.add)

                # combT [16, CH] (f32r) via matmul with bf16 I128
                pcT = ps_c.tile([E, CH], F32, tag="pcT")
                for j in range(4):
                    nc.tensor.matmul(pcT[:, j * 128:(j + 1) * 128],
                                     comb[:, j * E:(j + 1) * E],
                                     I128_t[:], start=True, stop=True)
                cT = sp.tile([E, CH], F32R, tag="cT")
                nc.vector.tensor_copy(cT[:], pcT[:])
                return cT, xtr

            def back(c, cT, xtr):
                """Expert MLP, weighted combine, store."""
                col0 = c * CH
                # per-h-row comb factor via SEL matmul (h-row = h*16 + e):
                # pce[p, t] = cT[p % 16, t]
                pce = ps_e.tile([128, CH], F32, tag="pce")
                nc.tensor.matmul(pce[:], SEL_t[:, :128], cT[:],
                                 start=True, stop=True)

                po_full = ps_a.tile([E, CH], F32, tag="pa")
                po = po_full[:O, :]
                for n in range(NH):
                    ph = ps_h.tile([128, CH], F32, tag="ph")
                    for k in range(NK):
                        nc.tensor.matmul(
                            ph[:], W1_t[k][:, n * 128:(n + 1) * 128],
                            xtr[k], start=(k == 0), stop=(k == NK - 1))
                    hsb = sp.tile([128, CH], F32, tag="hsb")
                    nc.scalar.activation(hsb[:], ph[:], ACTF.Relu)
                    g = sp.tile([128, CH], F32R, tag="g")
                    nc.vector.tensor_tensor(g[:], hsb[:], pce[:],
                                            op=ALU.mult)
                    nc.tensor.matmul(po[:], W2_t[n][:], g[:],
                                     start=(n == 0), stop=False)
                nc.tensor.matmul(po[:], W2b[:], cT[:], start=False, stop=True)

                # store transposed output [10, CH]; host untransposes
                osb = sp.tile([O, CH], F32, tag="osb")
                nc.scalar.activation(osb[:], po[:], ACTF.Copy)
                nc.sync.dma_start(out_d[:, col0:col0 + CH], osb[:])

            def body(rep):
                state = None
                for c in range(NCHUNK):
                    new = front(c)
                    if state is not None:
                        back(c - 1, *state)
                    state = new
                back(NCHUNK - 1, *state)

            if loop_reps > 1:
                with tc.For_i(0, loop_reps, 1) as _i:
                    body(_i)
            else:
                body(0)

    nc.compile()
    return nc


def _host_prep(x, gate_W, gate_b, W1, b1, W2, b2):
    x = np.asarray(x, np.float32)
    # xA[core, chunk, p, k*CH+j] = x[core*BL + chunk*CH + j, k*128+p], k<6
    xA = np.ascontiguousarray(
        x[:, :768].reshape(NCORES, NCHUNK, CH, 6, 128).transpose(0, 1, 4, 3, 2)
    ).reshape(NCORES, NCHUNK, 128, 6 * CH)
    # xB: d in [768,784) plus ones row (bias)
    xB = np.empty((NCORES, NCHUNK, DP - 768, CH), np.float32)
    xB[:, :, :D - 768, :] = x[:, 768:].reshape(
        NCORES, NCHUNK, CH, D - 768).transpose(0, 1, 3, 2)
    xB[:, :, D - 768:, :] = 1.0
    Wg = np.concatenate([np.asarray(gate_W, np.float32),
                         np.asarray(gate_b, np.float32)[None, :]], 0)
    Wgh = Wg.astype(np.float16)
    Wgl = ((Wg - Wgh.astype(np.float32)) * 2.0 ** 11).astype(np.float16)
    xAh = xA.astype(np.float16)
    xAl = ((xA - xAh.astype(np.float32)) * 2.0 ** 11).astype(np.float16)
    xBh = xB.astype(np.float16)
    xBl = ((xB - xBh.astype(np.float32)) * 2.0 ** 11).astype(np.float16)
    # expert-interleaved hidden layout: h-row index = h*E + e
    W1f = np.asarray(W1, np.float32).transpose(1, 2, 0).reshape(D, EH)
    W1a = np.concatenate([W1f, np.asarray(b1, np.float32).T.reshape(1, EH)], 0)
    W2a = np.concatenate([np.asarray(W2, np.float32).transpose(1, 0, 2).reshape(EH, O),
                          np.asarray(b2, np.float32)], 0)
    SEL = np.zeros((E, EH), np.float32)
    for r in range(EH):
        SEL[r % E, r] = 1.0
    import ml_dtypes
    consts = {
        "Wgh": Wgh, "Wgl": Wgl, "W1a": W1a.astype(np.float16),
        "W2a": W2a, "SEL": SEL,
        "I16": np.eye(E, dtype=np.float32),
        "I128": np.eye(128, dtype=np.float32).astype(ml_dtypes.bfloat16),
    }
    return (xAh, xAl), (xBh, xBl), consts


def kernel(x, gate_W, gate_b, W1, b1, W2, b2, _loop_reps=1):
    if _loop_reps not in _CACHED:
        _CACHED[_loop_reps] = _build_program(_loop_reps)
    nc = _CACHED[_loop_reps]
    xA, xB, consts = _host_prep(x, gate_W, gate_b, W1, b1, W2, b2)
    in_maps = []
    for cidx in range(NCORES):
        m = dict(consts)
        m["xAh"] = xA[0][cidx]
        m["xAl"] = xA[1][cidx]
        m["xBh"] = np.ascontiguousarray(xB[0][cidx])
        m["xBl"] = np.ascontiguousarray(xB[1][cidx])
        in_maps.append(m)
    res = run_bass_kernel_spmd(nc, in_maps, list(range(NCORES)))
    outT = np.concatenate([res.results[i]["out"] for i in range(NCORES)], 1)
    return np.ascontiguousarray(outT.T).astype(np.float32)
